# revision 15
# baseline (speedup 1.0000x reference)
"""L1-distance (vq_codebook) Trainium2 kernel.

Computes distances[b, v] = sum_d |x[b, d] - p[v, d]| for x (4096, 64),
p (1024, 64), plus r1 = mean_v min_b distances and r2 = mean_b min_v.

Strategy (8 cores, data-parallel over batch; 512 rows/core):

  |z| = 2*relu(z) - z  with z = p - x, so
  dist[b,v] = 2*sum_d relu(p_vd - x_bd) - P_v + X_b,
  P_v = sum_d p_vd, X_b = sum_d x_bd.

  - d-axis on SBUF partitions, two batch rows packed per 128 partitions:
      pT2    = [p.T ; p.T]                   (128, 1024) fp32, replicated
      negxT2 = [-xs[0:256].T ; -xs[256:].T]  (128, 256)  fp32, per shard
  - per pair k one fused relu op -> fp16 tile (128, 1024):
      DVE: tensor_scalar (pT2 + negxT2[:,k]) max 0   (2x fp32 mode)
      ACT: activation Relu with per-partition bias
  - sum over d on TensorE: stationary W_q (128,128) fp16, zero except
    2.0-blocks in columns 2q (parts 0-63) / 2q+1 (64-127); 64 accumulating
    matmuls + one extra matmul (stationary 0.5, moving -P_v/64 replicated)
    fill a (128, 512) PSUM bank with 2*sum(relu) - P_v.
  - PSUM -> SBUF copy adds X_b as per-partition scalar; DMA out in
    "hw row order"; host inverse-permutes rows, computes r1/r2.
"""

import os

import numpy as np

import concourse.bacc as bacc
import concourse.mybir as mybir
from concourse.bass_utils import run_bass_kernel_spmd
from concourse.tile import TileContext

B, V, D = 4096, 1024, 64
NCORES = 8
BS = B // NCORES          # 512 batch rows per core
PAIRS = BS // 2           # 256 pairs per core
GROUPS = PAIRS // 64      # 4 psum groups of 64 pairs
FP32 = mybir.dt.float32
FP16 = mybir.dt.float16
W_COLS = 64 * 128         # 64 stationary blocks
M2_OFF = W_COLS           # -P_v/64 moving tile
W2_OFF = W_COLS + V       # 0.5 stationary for the P_v matmul
WM_COLS = W_COLS + V + 128

last_exec_time_ns = None


def _ensure_ntff_hook():
    """Install antenv.axon_hooks shim if the image's antenv lacks it.

    Only needed for KTRACE=1 profiling runs; the plain execute path
    never imports antenv.axon_hooks.
    """
    import sys
    import types

    try:
        import antenv.axon_hooks  # noqa: F401

        return
    except ImportError:
        pass
    mod = types.ModuleType("antenv.axon_hooks")
    state = {}

    def set_axon_ntff_profile_hook(h):
        state["h"] = h

    def get_axon_ntff_profile_hook():
        if "h" not in state:
            try:
                from trn_agent_boot.trn_boot import _ntff_profile_via_ctypes

                state["h"] = _ntff_profile_via_ctypes("/opt/axon/libaxon_pjrt.so")
            except Exception:
                state["h"] = None
        return state.get("h")

    mod.set_axon_ntff_profile_hook = set_axon_ntff_profile_hook
    mod.get_axon_ntff_profile_hook = get_axon_ntff_profile_hook
    sys.modules["antenv.axon_hooks"] = mod
    import antenv

    antenv.axon_hooks = mod


def _build_nc():
    nc = bacc.Bacc(target_bir_lowering=False)
    # xin = [pT2 | negxT2 | X2col(4)] packed so one DMA covers all fp32
    # inputs (TensorScalarPtr has limited sync-wait slots).
    xin_ext = nc.declare_dram_parameter("xin", [128, V + PAIRS + GROUPS], FP32, isOutput=False)
    wm_ext = nc.declare_dram_parameter("wm", [128, WM_COLS], FP16, isOutput=False)
    out_ext = nc.declare_dram_parameter("out", [BS, V], FP32, isOutput=True)

    with TileContext(nc) as tc:
        with (
            tc.tile_pool(name="const", bufs=1) as cpool,
            tc.tile_pool(name="absd", bufs=6) as apool,
            tc.tile_pool(name="ps", bufs=8, space="PSUM") as ppool,
            tc.tile_pool(name="ot", bufs=4) as opool,
        ):
            xin = cpool.tile([128, V + PAIRS + GROUPS], FP32, tag="xin")
            nc.sync.dma_start(out=xin, in_=xin_ext[:])
            pT2 = xin[:, 0:V]
            negx = xin[:, V : V + PAIRS]
            xcol = xin[:, V + PAIRS : V + PAIRS + GROUPS]
            wm = cpool.tile([128, WM_COLS], FP16, tag="wm")
            for c in range(8):
                lo = c * (WM_COLS // 8)
                hi = (c + 1) * (WM_COLS // 8)
                nc.sync.dma_start(out=wm[:, lo:hi], in_=wm_ext[:, lo:hi])

            for g in range(GROUPS):
                ps = [
                    ppool.tile([128, 512], FP32, tag="ps", name=f"ps_{g}_{h}")
                    for h in range(2)
                ]
                for q in range(64):
                    k = g * 64 + q
                    absd = apool.tile([128, V], FP16, tag="absd")
                    if k % 3 == 2:
                        nc.scalar.activation(
                            out=absd,
                            in_=pT2,
                            func=mybir.ActivationFunctionType.Relu,
                            bias=negx[:, k : k + 1],
                            scale=1.0,
                        )
                    else:
                        nc.vector.tensor_scalar(
                            out=absd,
                            in0=pT2,
                            scalar1=negx[:, k : k + 1],
                            scalar2=0.0,
                            op0=mybir.AluOpType.add,
                            op1=mybir.AluOpType.max,
                        )
                    for h in range(2):
                        nc.tensor.matmul(
                            out=ps[h][:, :],
                            lhsT=wm[:, q * 128 : (q + 1) * 128],
                            rhs=absd[:, h * 512 : (h + 1) * 512],
                            start=(q == 0),
                            stop=False,
                        )
                for h in range(2):
                    # accumulate -P_v into every psum row
                    nc.tensor.matmul(
                        out=ps[h][:, :],
                        lhsT=wm[:, W2_OFF : W2_OFF + 128],
                        rhs=wm[:, M2_OFF + h * 512 : M2_OFF + (h + 1) * 512],
                        start=False,
                        stop=True,
                    )
                for h in range(2):
                    ot = opool.tile([128, 512], FP32, tag="ot")
                    if h == 0:
                        nc.scalar.activation(
                            out=ot,
                            in_=ps[h],
                            func=mybir.ActivationFunctionType.Identity,
                            bias=xcol[:, g : g + 1],
                            scale=1.0,
                        )
                    else:
                        nc.vector.tensor_scalar_add(
                            out=ot, in0=ps[h], scalar1=xcol[:, g : g + 1]
                        )
                    nc.sync.dma_start(
                        out=out_ext[g * 128 : (g + 1) * 128, h * 512 : (h + 1) * 512],
                        in_=ot,
                    )
    nc.finalize()
    return nc


def _b_of_hwrow() -> np.ndarray:
    r = np.arange(BS)
    g = r // 128
    q = (r % 128) // 2
    return g * 64 + q + 256 * (r % 2)


def _wm_sb(p: np.ndarray) -> np.ndarray:
    W = np.zeros((64, 128, 128), np.float16)
    for q in range(64):
        W[q, 0:64, 2 * q] = 2.0
        W[q, 64:128, 2 * q + 1] = 2.0
    w_blocks = W.transpose(1, 0, 2).reshape(128, W_COLS)
    P_v = p.astype(np.float64).sum(axis=1)
    m2 = np.broadcast_to((-P_v / 64.0).astype(np.float16), (128, V))
    w2 = np.full((128, 128), 0.5, np.float16)
    return np.ascontiguousarray(np.hstack([w_blocks, m2, w2]).astype(np.float16))


_nc_cache = {}


def kernel(x, trainable_p):
    global last_exec_time_ns
    x = np.ascontiguousarray(np.asarray(x, dtype=np.float32))
    p = np.ascontiguousarray(np.asarray(trainable_p, dtype=np.float32))
    assert x.shape == (B, D) and p.shape == (V, D)

    if "nc" not in _nc_cache:
        _nc_cache["nc"] = _build_nc()
    nc = _nc_cache["nc"]

    pT2 = np.vstack([p.T, p.T]).astype(np.float32)
    wm = _wm_sb(p)
    border = _b_of_hwrow()
    in_maps = []
    for c in range(NCORES):
        xs = x[c * BS : (c + 1) * BS]
        negx = np.vstack([-xs[0:PAIRS].T, -xs[PAIRS:].T]).astype(np.float32)
        X_hw = xs.astype(np.float64).sum(axis=1)[border].astype(np.float32)
        xcol = X_hw.reshape(GROUPS, 128).T  # [q, g]
        xin = np.ascontiguousarray(np.hstack([pT2, negx, xcol]).astype(np.float32))
        in_maps.append({"xin": xin, "wm": wm})

    trace = os.environ.get("KTRACE", "0") == "1"
    if trace:
        _ensure_ntff_hook()
    res = run_bass_kernel_spmd(nc, in_maps, list(range(NCORES)), trace=trace)
    last_exec_time_ns = res.exec_time_ns

    dist = np.empty((B, V), np.float32)
    for c in range(NCORES):
        hw = res.results[c]["out"]
        dist[c * BS + border] = hw

    r1 = np.float32(dist.min(axis=0).mean(dtype=np.float64))
    r2 = np.float32(dist.min(axis=1).mean(dtype=np.float64))
    return dist, r1, r2


# revision 17
# speedup vs baseline: 1.0022x; 1.0022x over previous
"""L1-distance (vq_codebook) Trainium2 kernel.

Computes distances[b, v] = sum_d |x[b, d] - p[v, d]| for x (4096, 64),
p (1024, 64), plus r1 = mean_v min_b distances and r2 = mean_b min_v.

Strategy (8 cores, data-parallel over batch; 512 rows/core):

  |z| = 2*relu(z) - z  with z = p - x, so
  dist[b,v] = 2*sum_d relu(p_vd - x_bd) - P_v + X_b,
  P_v = sum_d p_vd, X_b = sum_d x_bd.

  - d-axis on SBUF partitions, two batch rows packed per 128 partitions:
      pT2    = [p.T ; p.T]                   (128, 1024) fp32, replicated
      negxT2 = [-xs[0:256].T ; -xs[256:].T]  (128, 256)  fp32, per shard
  - per pair k one fused relu op -> fp16 tile (128, 1024):
      DVE: tensor_scalar (pT2 + negxT2[:,k]) max 0   (2x fp32 mode)
      ACT: activation Relu with per-partition bias
  - sum over d on TensorE: stationary W_q (128,128) fp16, zero except
    2.0-blocks in columns 2q (parts 0-63) / 2q+1 (64-127); 64 accumulating
    matmuls + one extra matmul (stationary 0.5, moving -P_v/64 replicated)
    fill a (128, 512) PSUM bank with 2*sum(relu) - P_v.
  - PSUM -> SBUF copy adds X_b as per-partition scalar; DMA out in
    "hw row order"; host inverse-permutes rows, computes r1/r2.
"""

import os

import numpy as np

import concourse.bacc as bacc
import concourse.mybir as mybir
from concourse.bass_utils import run_bass_kernel_spmd
from concourse.tile import TileContext

B, V, D = 4096, 1024, 64
NCORES = 8
BS = B // NCORES          # 512 batch rows per core
PAIRS = BS // 2           # 256 pairs per core
GROUPS = PAIRS // 64      # 4 psum groups of 64 pairs
FP32 = mybir.dt.float32
FP16 = mybir.dt.float16
W_COLS = 64 * 128         # 64 stationary blocks
M2_OFF = W_COLS           # -P_v/64 moving tile
W2_OFF = W_COLS + V       # 0.5 stationary for the P_v matmul
WM_COLS = W_COLS + V + 128

last_exec_time_ns = None
last_results = None


def _ensure_ntff_hook():
    """Install antenv.axon_hooks shim if the image's antenv lacks it.

    Only needed for KTRACE=1 profiling runs; the plain execute path
    never imports antenv.axon_hooks.
    """
    import sys
    import types

    try:
        import antenv.axon_hooks  # noqa: F401

        return
    except ImportError:
        pass
    mod = types.ModuleType("antenv.axon_hooks")
    state = {}

    def set_axon_ntff_profile_hook(h):
        state["h"] = h

    def get_axon_ntff_profile_hook():
        if "h" not in state:
            try:
                from trn_agent_boot.trn_boot import _ntff_profile_via_ctypes

                state["h"] = _ntff_profile_via_ctypes("/opt/axon/libaxon_pjrt.so")
            except Exception:
                state["h"] = None
        return state.get("h")

    mod.set_axon_ntff_profile_hook = set_axon_ntff_profile_hook
    mod.get_axon_ntff_profile_hook = get_axon_ntff_profile_hook
    sys.modules["antenv.axon_hooks"] = mod
    import antenv

    antenv.axon_hooks = mod


def _build_nc():
    nc = bacc.Bacc(target_bir_lowering=False)
    # xin = [pT2 | negxT2 | X2col(4)] packed so one DMA covers all fp32
    # inputs (TensorScalarPtr has limited sync-wait slots).
    xin_ext = nc.declare_dram_parameter("xin", [128, V + PAIRS + GROUPS], FP32, isOutput=False)
    wm_ext = nc.declare_dram_parameter("wm", [128, WM_COLS], FP16, isOutput=False)
    out_ext = nc.declare_dram_parameter("out", [BS, V], FP32, isOutput=True)

    with TileContext(nc) as tc:
        with (
            tc.tile_pool(name="const", bufs=1) as cpool,
            tc.tile_pool(name="absd", bufs=6) as apool,
            tc.tile_pool(name="ps", bufs=8, space="PSUM") as ppool,
            tc.tile_pool(name="ot", bufs=4) as opool,
        ):
            xin = cpool.tile([128, V + PAIRS + GROUPS], FP32, tag="xin")
            nc.sync.dma_start(out=xin, in_=xin_ext[:])
            pT2 = xin[:, 0:V]
            negx = xin[:, V : V + PAIRS]
            xcol = xin[:, V + PAIRS : V + PAIRS + GROUPS]
            wm = cpool.tile([128, WM_COLS], FP16, tag="wm")
            for c in range(8):
                lo = c * (WM_COLS // 8)
                hi = (c + 1) * (WM_COLS // 8)
                nc.sync.dma_start(out=wm[:, lo:hi], in_=wm_ext[:, lo:hi])

            for g in range(GROUPS):
                ps = [
                    ppool.tile([128, 512], FP32, tag="ps", name=f"ps_{g}_{h}")
                    for h in range(2)
                ]
                for q in range(64):
                    k = g * 64 + q
                    absd = apool.tile([128, V], FP16, tag="absd")
                    if k % 3 == 2:
                        nc.scalar.activation(
                            out=absd,
                            in_=pT2,
                            func=mybir.ActivationFunctionType.Relu,
                            bias=negx[:, k : k + 1],
                            scale=1.0,
                        )
                    else:
                        nc.vector.tensor_scalar(
                            out=absd,
                            in0=pT2,
                            scalar1=negx[:, k : k + 1],
                            scalar2=0.0,
                            op0=mybir.AluOpType.add,
                            op1=mybir.AluOpType.max,
                        )
                    for h in range(2):
                        nc.tensor.matmul(
                            out=ps[h][:, :],
                            lhsT=wm[:, q * 128 : (q + 1) * 128],
                            rhs=absd[:, h * 512 : (h + 1) * 512],
                            start=(q == 0),
                            stop=False,
                        )
                for h in range(2):
                    # accumulate -P_v into every psum row
                    nc.tensor.matmul(
                        out=ps[h][:, :],
                        lhsT=wm[:, W2_OFF : W2_OFF + 128],
                        rhs=wm[:, M2_OFF + h * 512 : M2_OFF + (h + 1) * 512],
                        start=False,
                        stop=True,
                    )
                for h in range(2):
                    ot = opool.tile([128, 512], FP32, tag="ot")
                    if h == 0:
                        nc.scalar.activation(
                            out=ot,
                            in_=ps[h],
                            func=mybir.ActivationFunctionType.Identity,
                            bias=xcol[:, g : g + 1],
                            scale=1.0,
                        )
                    else:
                        nc.vector.tensor_scalar_add(
                            out=ot, in0=ps[h], scalar1=xcol[:, g : g + 1]
                        )
                    nc.sync.dma_start(
                        out=out_ext[g * 128 : (g + 1) * 128, h * 512 : (h + 1) * 512],
                        in_=ot,
                    )
    nc.finalize()
    return nc


def _b_of_hwrow() -> np.ndarray:
    r = np.arange(BS)
    g = r // 128
    q = (r % 128) // 2
    return g * 64 + q + 256 * (r % 2)


def _wm_sb(p: np.ndarray) -> np.ndarray:
    W = np.zeros((64, 128, 128), np.float16)
    for q in range(64):
        W[q, 0:64, 2 * q] = 2.0
        W[q, 64:128, 2 * q + 1] = 2.0
    w_blocks = W.transpose(1, 0, 2).reshape(128, W_COLS)
    P_v = p.astype(np.float64).sum(axis=1)
    m2 = np.broadcast_to((-P_v / 64.0).astype(np.float16), (128, V))
    w2 = np.full((128, 128), 0.5, np.float16)
    return np.ascontiguousarray(np.hstack([w_blocks, m2, w2]).astype(np.float16))


_nc_cache = {}


def kernel(x, trainable_p):
    global last_exec_time_ns
    x = np.ascontiguousarray(np.asarray(x, dtype=np.float32))
    p = np.ascontiguousarray(np.asarray(trainable_p, dtype=np.float32))
    assert x.shape == (B, D) and p.shape == (V, D)

    if "nc" not in _nc_cache:
        _nc_cache["nc"] = _build_nc()
    nc = _nc_cache["nc"]

    pT2 = np.vstack([p.T, p.T]).astype(np.float32)
    wm = _wm_sb(p)
    border = _b_of_hwrow()
    in_maps = []
    for c in range(NCORES):
        xs = x[c * BS : (c + 1) * BS]
        negx = np.vstack([-xs[0:PAIRS].T, -xs[PAIRS:].T]).astype(np.float32)
        X_hw = xs.astype(np.float64).sum(axis=1)[border].astype(np.float32)
        xcol = X_hw.reshape(GROUPS, 128).T  # [q, g]
        xin = np.ascontiguousarray(np.hstack([pT2, negx, xcol]).astype(np.float32))
        in_maps.append({"xin": xin, "wm": wm})

    trace = os.environ.get("KTRACE", "0") == "1"
    if trace:
        _ensure_ntff_hook()
    res = run_bass_kernel_spmd(nc, in_maps, list(range(NCORES)), trace=trace)
    last_exec_time_ns = res.exec_time_ns
    global last_results
    last_results = res

    dist = np.empty((B, V), np.float32)
    for c in range(NCORES):
        hw = res.results[c]["out"]
        dist[c * BS + border] = hw

    r1 = np.float32(dist.min(axis=0).mean(dtype=np.float64))
    r2 = np.float32(dist.min(axis=1).mean(dtype=np.float64))
    return dist, r1, r2
